# revision 11
# baseline (speedup 1.0000x reference)
"""Distributed Trainium2 Bass kernel for a dense pre-LN transformer block.

Problem: x:[4,2048,1024] f32; per-head QKV (H=16, HS=64), causal attention,
out-proj + residual, pre-LN MLP (4x) + residual.

Sharding over 8 NeuronCores:
- Tokens (B*T = 8192) are sharded 8x1024 for LN1/out-proj/LN2/MLP (data
  parallel over flattened tokens).
- Attention is head-sharded: each core computes 2 heads over all 8192 tokens.
- Two bf16 collectives connect the layouts: an AllGather of the transposed
  LN1 output (2 MB/rank) and an AllToAll of the transposed attention output
  (2 MB/rank).  The final output gather is done host-side (free).

All matmuls run in bf16 (4x fp32 PE throughput) with f32 PSUM accumulation;
LN / softmax / residual arithmetic stays in f32.  Softmax skips the max
subtraction (scores are provably O(1) here) and gets its denominator from a
ones-column appended to V in the same accumulating matmul.

SBUF is managed with nested (LIFO) tile-pool scopes; long-lived tensors that
come alive late (x2, h2^T, relu activations) go on the right-side stack so
their lifetimes need not nest with the attention-phase pools.
"""

import numpy as np
import ml_dtypes

import concourse.bass as bass
import concourse.bacc as bacc
import concourse.tile as tile
import concourse.mybir as mybir
from concourse.bass_utils import run_bass_kernel_spmd
from concourse.masks import make_identity, make_upper_triangular

BF16 = mybir.dt.bfloat16
F32 = mybir.dt.float32
NP_BF16 = ml_dtypes.bfloat16
P = 128
EPS = 1e-5


class Cfg:
    def __init__(self, B=4, T=2048, D=1024, DH=4096, HS=64, NC=8):
        self.B, self.T, self.D, self.DH, self.HS, self.NC = B, T, D, DH, HS, NC
        self.H = D // HS                  # total heads
        self.HPC = self.H // NC           # heads per core
        assert self.HPC * HS * NC == D    # feature rows == NC * P when HPC*HS==P
        assert self.HPC * HS == P
        self.TOK = B * T                  # global tokens
        self.TSH = self.TOK // NC         # tokens per core
        self.NTT = self.TSH // P          # 128-token chunks per core
        self.DC = D // P                  # dim chunks
        self.HC = DH // P                 # hidden chunks
        self.NUC = T // P                 # key chunks per (b, h)
        self.RPB = T // self.TSH          # ranks owning one batch's tokens
        assert self.T % 512 == 0 and self.TSH % P == 0 and D % P == 0
        assert self.T % self.TSH == 0


FULL = Cfg()
SMALL = Cfg(B=4, T=512, D=1024, DH=1024)


def build_nc(cfg: Cfg):
    nc = bacc.Bacc("TRN2", target_bir_lowering=False, debug=False,
                   num_devices=cfg.NC)
    B, T, D, DH, HS, NC = cfg.B, cfg.T, cfg.D, cfg.DH, cfg.HS, cfg.NC
    TOK, TSH, NTT, DC, HC, NUC, HPC, RPB = (
        cfg.TOK, cfg.TSH, cfg.NTT, cfg.DC, cfg.HC, cfg.NUC, cfg.HPC, cfg.RPB)
    rg = [list(range(NC))]

    def segs(n, w=512):
        return [(s, min(n, s + w)) for s in range(0, n, w)]

    # ---- parameters (per-core shards supplied host-side) ----
    x_ext = nc.declare_dram_parameter("x", [TSH, D], F32, isOutput=False)
    wq_ext = nc.declare_dram_parameter("wq", [D, P], BF16, isOutput=False)
    wk_ext = nc.declare_dram_parameter("wk", [D, P], BF16, isOutput=False)
    wv_ext = nc.declare_dram_parameter("wv", [D, P], BF16, isOutput=False)
    wo_ext = nc.declare_dram_parameter("wo", [D, D], BF16, isOutput=False)
    w1_ext = nc.declare_dram_parameter("w1", [D, DH], BF16, isOutput=False)
    w2_ext = nc.declare_dram_parameter("w2", [DH, D], BF16, isOutput=False)
    g1_ext = nc.declare_dram_parameter("g1", [1, D], F32, isOutput=False)
    be1_ext = nc.declare_dram_parameter("be1", [1, D], F32, isOutput=False)
    g2_ext = nc.declare_dram_parameter("g2", [1, D], F32, isOutput=False)
    be2_ext = nc.declare_dram_parameter("be2", [1, D], F32, isOutput=False)
    bo_ext = nc.declare_dram_parameter("bo", [1, D], F32, isOutput=False)
    b2_ext = nc.declare_dram_parameter("b2", [1, D], F32, isOutput=False)
    b1t_ext = nc.declare_dram_parameter("b1t", [P, HC], F32, isOutput=False)
    out_ext = nc.declare_dram_parameter("out", [TSH, D], F32, isOutput=True)

    # ---- internal DRAM (collective bounce buffers) ----
    h1t_bounce = nc.dram_tensor("h1t_bounce", [D, TSH], BF16)
    h1t_full = nc.dram_tensor("h1t_full", [NC * D, TSH], BF16,
                              addr_space="Shared")
    att_bounce = nc.dram_tensor("att_bounce", [NC * P, TSH], BF16)
    att_a2a = nc.dram_tensor("att_a2a", [NC * P, TSH], BF16)

    def bcast_row(handle):
        return bass.AP(tensor=handle, offset=0, ap=[[0, P], [1, D]])

    with tile.TileContext(nc) as tc:
        with tc.tile_pool(name="const", bufs=1) as const, \
             tc.tile_pool(name="ln", bufs=2) as ln_pool:
            ident = const.tile([P, P], BF16)
            make_identity(nc, ident)
            # HS-identity replicated at each head's base partition, so the
            # V transpose (lhsT at base h0) has a same-base rhs.
            ident2 = const.tile([P, HS], BF16)
            for hl in range(HPC):
                nc.sync.dma_start(out=ident2[hl * HS:(hl + 1) * HS, :],
                                  in_=ident[0:HS, 0:HS])
            tri = const.tile([P, P], BF16)      # tri[u, t] = 1 iff u <= t
            make_upper_triangular(nc, tri, val=1.0, diag=True)
            eps_t = const.tile([P, 1], F32)
            nc.vector.memset(eps_t, EPS)
            g1_sb = const.tile([P, D], F32)
            nc.sync.dma_start(out=g1_sb, in_=bcast_row(g1_ext))
            be1_sb = const.tile([P, D], F32)
            nc.sync.dma_start(out=be1_sb, in_=bcast_row(be1_ext))
            g2_sb = const.tile([P, D], F32)
            nc.sync.dma_start(out=g2_sb, in_=bcast_row(g2_ext))
            be2_sb = const.tile([P, D], F32)
            nc.sync.dma_start(out=be2_sb, in_=bcast_row(be2_ext))
            bo_sb = const.tile([P, D], F32)
            nc.sync.dma_start(out=bo_sb, in_=bcast_row(bo_ext))
            b2_sb = const.tile([P, D], F32)
            nc.sync.dma_start(out=b2_sb, in_=bcast_row(b2_ext))
            b1t_sb = const.tile([P, HC], F32)
            nc.sync.dma_start(out=b1t_sb, in_=b1t_ext[:])
            wq_sb = const.tile([P, DC, P], BF16)
            nc.sync.dma_start(out=wq_sb, in_=wq_ext[:].rearrange(
                "(dc p) m -> p dc m", p=P))
            wk_sb = const.tile([P, DC, P], BF16)
            nc.sync.dma_start(out=wk_sb, in_=wk_ext[:].rearrange(
                "(dc p) m -> p dc m", p=P))
            wv_sb = const.tile([P, DC, P], BF16)
            nc.sync.dma_start(out=wv_sb, in_=wv_ext[:].rearrange(
                "(dc p) m -> p dc m", p=P))

            def layernorm(src_ap, g_sb, b_sb, dst_bf):
                """LN over free axis D of [P, D] f32 src -> bf16 dst tile."""
                stats = ln_pool.tile([P, D // 512, 6], F32, tag="stats")
                for s in range(D // 512):
                    nc.vector.bn_stats(out=stats[:, s, :],
                                       in_=src_ap[:, s * 512:(s + 1) * 512])
                mv = ln_pool.tile([P, 2], F32, tag="mv")
                nc.vector.bn_aggr(out=mv, in_=stats)
                std = ln_pool.tile([P, 1], F32, tag="std")
                nc.scalar.activation(out=std, in_=mv[:, 1:2],
                                     func=mybir.ActivationFunctionType.Sqrt,
                                     bias=eps_t)
                rstd = ln_pool.tile([P, 1], F32, tag="rstd")
                nc.vector.reciprocal(out=rstd, in_=std)
                tmp = ln_pool.tile([P, D], F32, tag="lntmp")
                nc.vector.tensor_scalar(out=tmp, in0=src_ap,
                                        scalar1=mv[:, 0:1], scalar2=rstd,
                                        op0=mybir.AluOpType.subtract,
                                        op1=mybir.AluOpType.mult)
                nc.vector.tensor_mul(out=tmp, in0=tmp, in1=g_sb)
                nc.vector.tensor_add(out=dst_bf, in0=tmp, in1=b_sb)

            # x2 (post-attention residual stream) lives from phase 4 to the
            # end; allocate on the right-side stack so the attention-phase
            # pools (left) can be released out from under it.
            with tc.tile_pool(name="resid", bufs=1, side="right") as resid:
                x2_sb = resid.tile([P, NTT, D], F32)

                # ======== Phase 1: LN1 + transpose + AllGather ========
                with tc.tile_pool(name="xin", bufs=3) as xin, \
                     tc.tile_pool(name="h1tp", bufs=1) as h1tp:
                    h1t_sb = h1tp.tile([P, DC, TSH], BF16)
                    with tc.tile_pool(name="tr_psum", bufs=2,
                                      space="PSUM") as trp:
                        for i in range(NTT):
                            x_t = xin.tile([P, D], F32, tag="x")
                            nc.sync.dma_start(
                                out=x_t, in_=x_ext[i * P:(i + 1) * P, :])
                            h1_bf = ln_pool.tile([P, D], BF16, tag="h1bf")
                            layernorm(x_t, g1_sb, be1_sb, h1_bf)
                            for dc in range(DC):
                                pt = trp.tile([P, P], BF16)
                                nc.tensor.transpose(
                                    pt, h1_bf[:, dc * P:(dc + 1) * P], ident)
                                nc.vector.tensor_copy(
                                    out=h1t_sb[:, dc, i * P:(i + 1) * P],
                                    in_=pt)
                    nc.sync.dma_start(out=h1t_bounce[:].rearrange(
                        "(dc p) t -> p dc t", p=P), in_=h1t_sb)
                    nc.gpsimd.collective_compute(
                        "AllGather", mybir.AluOpType.bypass, replica_groups=rg,
                        ins=[h1t_bounce[:]], outs=[h1t_full[:]])

                # ======== Phases 2+3: QKV then attention ========
                with tc.tile_pool(name="qkvp", bufs=1) as qkvp:
                    qt_sb = qkvp.tile([P, TOK], BF16)
                    kt_sb = qkvp.tile([P, TOK], BF16)
                    vt_sb = qkvp.tile([P, TOK], BF16)
                    hview = h1t_full[:].rearrange(
                        "(r dc p) t -> r dc p t", dc=DC, p=P)
                    with tc.tile_pool(name="h1in", bufs=10) as h1in, \
                         tc.tile_pool(name="qkv_psum", bufs=2,
                                      space="PSUM") as qp:
                        for r in range(NC):
                            tiles = []
                            for dc in range(DC):
                                ht = h1in.tile([P, TSH], BF16, tag="h1t")
                                nc.sync.dma_start(out=ht, in_=hview[r, dc])
                                tiles.append(ht)
                            for w_sb, dst, eng in ((wq_sb, qt_sb, "v"),
                                                   (wk_sb, kt_sb, "s"),
                                                   (wv_sb, vt_sb, "v")):
                                ps = qp.tile([P, TSH], F32, tag="ps")
                                for dc in range(DC):
                                    for (s0, s1) in segs(TSH):
                                        nc.tensor.matmul(
                                            ps[:, s0:s1],
                                            lhsT=w_sb[:, dc, :],
                                            rhs=tiles[dc][:, s0:s1],
                                            start=(dc == 0),
                                            stop=(dc == DC - 1))
                                dslice = dst[:, r * TSH:(r + 1) * TSH]
                                if eng == "s":
                                    nc.scalar.copy(out=dslice, in_=ps)
                                else:
                                    nc.vector.tensor_copy(out=dslice, in_=ps)

                    # -------- attention per (batch, local head) --------
                    with tc.tile_pool(name="apool", bufs=2) as apool, \
                         tc.tile_pool(name="epool", bufs=4) as epool, \
                         tc.tile_pool(name="dpool", bufs=1) as dpool, \
                         tc.tile_pool(name="sc_psum", bufs=2,
                                      space="PSUM") as scp, \
                         tc.tile_pool(name="av_psum", bufs=1,
                                      space="PSUM") as avp, \
                         tc.tile_pool(name="vt_psum", bufs=2,
                                      space="PSUM") as vtp:
                        for b in range(B):
                            for hl in range(HPC):
                                h0 = hl * HS
                                base = b * T
                                # V token-major [u, 64] + ones column
                                v_sb = apool.tile([P, NUC, HS + 1], BF16,
                                                  tag="v")
                                nc.vector.memset(v_sb[:, :, HS:HS + 1], 1.0)
                                for uc in range(NUC):
                                    pv = vtp.tile([P, HS], BF16, tag="pv")
                                    nc.tensor.transpose(
                                        pv,
                                        vt_sb[h0:h0 + HS,
                                              base + uc * P:base + (uc + 1) * P],
                                        ident2[h0:h0 + HS, :])
                                    nc.vector.tensor_copy(
                                        out=v_sb[:, uc, 0:HS], in_=pv)
                                av = avp.tile([HS + 1, T], F32, tag="av")
                                for uc in range(NUC):
                                    t0 = uc * P
                                    k_lhsT = kt_sb[h0:h0 + HS,
                                                   base + t0:base + t0 + P]
                                    s = t0
                                    while s < T:
                                        e = min(T, (s // 512 + 1) * 512)
                                        w = e - s
                                        sp = scp.tile([P, 512], F32, tag="sc")
                                        nc.tensor.matmul(
                                            sp[:, 0:w], lhsT=k_lhsT,
                                            rhs=qt_sb[h0:h0 + HS,
                                                      base + s:base + e],
                                            start=True, stop=True)
                                        ex = epool.tile([P, 512], BF16,
                                                        tag="e")
                                        nc.scalar.activation(
                                            out=ex[:, 0:w], in_=sp[:, 0:w],
                                            func=mybir.ActivationFunctionType.Exp)
                                        if s == t0:  # diagonal: causal mask
                                            nc.vector.tensor_mul(
                                                out=ex[:, 0:P],
                                                in0=ex[:, 0:P], in1=tri)
                                        bk = s // 512
                                        nc.tensor.matmul(
                                            av[:, s:e], lhsT=v_sb[:, uc, :],
                                            rhs=ex[:, 0:w], start=(uc == 0),
                                            stop=(uc == min(NUC - 1,
                                                            4 * bk + 3)))
                                        s = e
                                # divide by the ones-row denominator
                                rcp = dpool.tile([1, T], F32, tag="rcp")
                                nc.vector.reciprocal(out=rcp,
                                                     in_=av[HS:HS + 1, :])
                                rb = dpool.tile([HS, T], F32, tag="rb")
                                nc.gpsimd.partition_broadcast(rb, rcp)
                                att_d = dpool.tile([HS, T], BF16, tag="att")
                                nc.vector.tensor_mul(out=att_d,
                                                     in0=av[0:HS, :], in1=rb)
                                for k in range(RPB):
                                    j = (b * T) // TSH + k
                                    nc.sync.dma_start(
                                        out=att_bounce[
                                            j * P + h0:j * P + h0 + HS, :],
                                        in_=att_d[:, k * TSH:(k + 1) * TSH])
                nc.gpsimd.collective_compute(
                    "AllToAll", mybir.AluOpType.bypass, replica_groups=rg,
                    ins=[att_bounce[:]], outs=[att_a2a[:]])

                # ======== Phase 4: out-proj + residual -> x2 ========
                aview = att_a2a[:].rearrange("(fc p) t -> p fc t", p=P)
                with tc.tile_pool(name="wop", bufs=1) as wop, \
                     tc.tile_pool(name="atin", bufs=3) as atin, \
                     tc.tile_pool(name="op_psum", bufs=2, space="PSUM") as opp:
                    wo_sb = wop.tile([P, DC, D], BF16)
                    nc.sync.dma_start(out=wo_sb, in_=wo_ext[:].rearrange(
                        "(dc p) n -> p dc n", p=P))
                    for tt in range(NTT):
                        a_sb = atin.tile([P, NC, P], BF16, tag="a")
                        nc.sync.dma_start(
                            out=a_sb, in_=aview[:, :, tt * P:(tt + 1) * P])
                        x_t = atin.tile([P, D], F32, tag="x")
                        nc.sync.dma_start(out=x_t,
                                          in_=x_ext[tt * P:(tt + 1) * P, :])
                        po = opp.tile([P, D], F32, tag="po")
                        for fc in range(NC):
                            for (s0, s1) in segs(D):
                                nc.tensor.matmul(
                                    po[:, s0:s1], lhsT=a_sb[:, fc, :],
                                    rhs=wo_sb[:, fc, s0:s1],
                                    start=(fc == 0), stop=(fc == NC - 1))
                        nc.vector.tensor_add(out=x2_sb[:, tt, :], in0=po,
                                             in1=x_t)
                        nc.vector.tensor_add(out=x2_sb[:, tt, :],
                                             in0=x2_sb[:, tt, :], in1=bo_sb)

                # ======== Phase 5: LN2 + transpose ========
                with tc.tile_pool(name="h2tp", bufs=1, side="right") as h2tp:
                    h2t_sb = h2tp.tile([P, DC, TSH], BF16)
                    with tc.tile_pool(name="tr2_psum", bufs=2,
                                      space="PSUM") as tr2:
                        for i in range(NTT):
                            h2_bf = ln_pool.tile([P, D], BF16, tag="h2bf")
                            layernorm(x2_sb[:, i, :], g2_sb, be2_sb, h2_bf)
                            for dc in range(DC):
                                pt = tr2.tile([P, P], BF16, tag="pt2")
                                nc.tensor.transpose(
                                    pt, h2_bf[:, dc * P:(dc + 1) * P], ident)
                                nc.vector.tensor_copy(
                                    out=h2t_sb[:, dc, i * P:(i + 1) * P],
                                    in_=pt)

                    # ======== Phase 6: MLP1 (relu(h2 @ W1 + b1)) ========
                    with tc.tile_pool(name="actp", bufs=1,
                                      side="right") as actp:
                        act_sb = actp.tile([P, HC, TSH], BF16)
                        w1view = w1_ext[:].rearrange(
                            "(dc p) (hc m) -> p dc hc m", p=P, m=P)
                        with tc.tile_pool(name="w1in", bufs=3) as w1in, \
                             tc.tile_pool(name="m1_psum", bufs=2,
                                          space="PSUM") as m1p:
                            for hc in range(HC):
                                w1t = w1in.tile([P, DC, P], BF16, tag="w1")
                                nc.sync.dma_start(out=w1t,
                                                  in_=w1view[:, :, hc, :])
                                pm = m1p.tile([P, TSH], F32, tag="pm")
                                for dc in range(DC):
                                    for (s0, s1) in segs(TSH):
                                        nc.tensor.matmul(
                                            pm[:, s0:s1], lhsT=w1t[:, dc, :],
                                            rhs=h2t_sb[:, dc, s0:s1],
                                            start=(dc == 0),
                                            stop=(dc == DC - 1))
                                nc.scalar.activation(
                                    out=act_sb[:, hc, :], in_=pm,
                                    func=mybir.ActivationFunctionType.Relu,
                                    bias=b1t_sb[:, hc:hc + 1])

                        # ======== Phase 7: MLP2 + residual -> out ========
                        w2view = w2_ext[:].rearrange("(hc p) n -> p hc n", p=P)
                        GRP = 4 if NTT % 4 == 0 else 2
                        with tc.tile_pool(name="w2in", bufs=3) as w2in, \
                             tc.tile_pool(name="opool", bufs=3) as opool, \
                             tc.tile_pool(name="m2_psum", bufs=1,
                                          space="PSUM") as m2p:
                            for g in range(NTT // GRP):
                                psums = [m2p.tile([P, D], F32,
                                                  name=f"m2ps{_t}")
                                         for _t in range(GRP)]
                                for hc in range(HC):
                                    w2t = w2in.tile([P, D], BF16, tag="w2")
                                    nc.sync.dma_start(out=w2t,
                                                      in_=w2view[:, hc, :])
                                    for ti in range(GRP):
                                        tt = g * GRP + ti
                                        for (s0, s1) in segs(D):
                                            nc.tensor.matmul(
                                                psums[ti][:, s0:s1],
                                                lhsT=act_sb[:, hc,
                                                            tt * P:(tt + 1) * P],
                                                rhs=w2t[:, s0:s1],
                                                start=(hc == 0),
                                                stop=(hc == HC - 1))
                                for ti in range(GRP):
                                    tt = g * GRP + ti
                                    o_sb = opool.tile([P, D], F32, tag="o")
                                    nc.vector.tensor_add(
                                        out=o_sb, in0=psums[ti],
                                        in1=x2_sb[:, tt, :])
                                    nc.vector.tensor_add(out=o_sb, in0=o_sb,
                                                         in1=b2_sb)
                                    nc.sync.dma_start(
                                        out=out_ext[tt * P:(tt + 1) * P, :],
                                        in_=o_sb)

    nc.finalize()
    return nc


def shard_inputs(cfg: Cfg, inputs):
    """Full inputs (reference layout) -> per-core in_maps in kernel layout."""
    B, T, D, DH, HS, NC, HPC = (cfg.B, cfg.T, cfg.D, cfg.DH, cfg.HS, cfg.NC,
                                cfg.HPC)
    f32 = np.float32
    x = np.asarray(inputs["x"], f32).reshape(cfg.TOK, D)
    Wq = np.asarray(inputs["Wq"], f32)
    Wk = np.asarray(inputs["Wk"], f32)
    Wv = np.asarray(inputs["Wv"], f32)
    Wo = np.ascontiguousarray(np.asarray(inputs["Wo"], f32)).astype(NP_BF16)
    W1 = np.ascontiguousarray(np.asarray(inputs["W1"], f32)).astype(NP_BF16)
    W2 = np.ascontiguousarray(np.asarray(inputs["W2"], f32)).astype(NP_BF16)
    row = lambda v: np.asarray(v, f32).reshape(1, D)
    g1, be1 = row(inputs["g1"]), np.asarray(inputs["be1"], f32).reshape(1, D)
    g2, be2 = row(inputs["g2"]), np.asarray(inputs["be2"], f32).reshape(1, D)
    bo, b2 = row(inputs["bo"]), row(inputs["b2"])
    b1t = np.ascontiguousarray(
        np.asarray(inputs["b1"], f32).reshape(cfg.HC, P).T)

    in_maps = []
    for c in range(NC):
        hs = slice(c * HPC, (c + 1) * HPC)
        wq = Wq[hs].transpose(1, 0, 2).reshape(D, HPC * HS) * (HS ** -0.5)
        wk = Wk[hs].transpose(1, 0, 2).reshape(D, HPC * HS)
        wv = Wv[hs].transpose(1, 0, 2).reshape(D, HPC * HS)
        in_maps.append({
            "x": np.ascontiguousarray(x[c * cfg.TSH:(c + 1) * cfg.TSH]),
            "wq": np.ascontiguousarray(wq).astype(NP_BF16),
            "wk": np.ascontiguousarray(wk).astype(NP_BF16),
            "wv": np.ascontiguousarray(wv).astype(NP_BF16),
            "wo": Wo, "w1": W1, "w2": W2,
            "g1": g1, "be1": be1, "g2": g2, "be2": be2,
            "bo": bo, "b2": b2, "b1t": b1t,
        })
    return in_maps


_cache = {}


def _get_nc(cfg: Cfg):
    key = (cfg.B, cfg.T, cfg.D, cfg.DH)
    if key not in _cache:
        _cache[key] = build_nc(cfg)
    return _cache[key]


def kernel(**inputs) -> np.ndarray:
    cfg = FULL
    nc = _get_nc(cfg)
    in_maps = shard_inputs(cfg, inputs)
    res = run_bass_kernel_spmd(nc, in_maps, core_ids=list(range(cfg.NC)))
    out = np.concatenate([res.results[c]["out"] for c in range(cfg.NC)],
                         axis=0)
    return out.reshape(cfg.B, cfg.T, cfg.D).astype(np.float32)
